# revision 56
# baseline (speedup 1.0000x reference)
"""DirectionalGINConv (eps=0) Trainium2 kernel, 8-core SPMD.

  agg_i = sum_{j->i} x_j ; out = relu(relu((x + agg) @ W.T + b))

Strategy (hardcoded for N=50000, E=800000, C=64, 8 cores):
  - Destination-node sharding: core c owns dst rows [c*6250, (c+1)*6250).
  - Host routes edges into per-(dst-block-of-128) tile groups of 128 edges.
    Each block b gets T_b gather tiles (uniform across cores for SPMD):
    the first K0_b tiles hold edges with src < 32768 (gather table base row
    0), the rest hold src >= 17232 (base row 17232), so gather indices fit
    in int16 (dma_gather limit). K0_b*128 is chosen so every core can route
    exactly K0_b*128 low-src edges to block b (zero pad in half0); only the
    half1 tail tile of each block carries pad slots.
  - Device per core: dma_gather x rows (fp16, rows padded to 128ch = 256B)
    block-grouped; two gather calls per chunk of blocks, round-robinned
    over all 4 SWDGE queues with all index tiles preloaded, so ~4 gather
    descriptor streams stay in flight (the SWDGE desc-gen CPU pair per
    queue is the bottleneck, not the DMA engines).
  - Segment-sum via PE: per block build one-hot S[e, slot, j] on DVE with
    packed last dims (2x DVE mode) in a single op, then T_b accumulating
    matmuls psum[ch, slot] += G_tile.T @ S[:, :, j]; h = psum + x_shard.T;
    MLP = W.T-stationary matmul; relu+bias on ACT; PE transpose back to
    node-major; DMA out.
"""

import numpy as np
from contextlib import ExitStack

import ml_dtypes

N_NODES = 50000
IN_CH = 64
OUT_CH = 64
N_CORES = 8
SHARD = N_NODES // N_CORES          # 6250
P = 128
NBLK = (SHARD + P - 1) // P         # 49 blocks (last has 106 slots)
BASE1 = 17232                       # half1 table base (50000 - 32768)
CHUNKS = [3] * 16 + [1]             # blocks per gather chunk (sum=49); one
                                    # gather call per (chunk, half), ~3-3.5k
                                    # indices (the SWDGE idx scratch slows
                                    # beyond ~4k/call)


def _route(src, dst):
    """Vectorized edge routing with uniform-across-cores variable tiling.

    Returns dict with:
      K0, K1: [NBLK] int arrays (tiles per half, uniform across cores)
      idx0, idx1: [N_CORES, L0], [N_CORES, L1] int16 gather indices
      slots: [N_CORES, LT] float32 slot-in-block (-1 pad), block-tile-major
    where L0 = sum(K0)*128, L1 = sum(K1)*128, LT = L0+L1.
    """
    src = np.asarray(src, np.int64)
    dst = np.asarray(dst, np.int64)
    core = dst // SHARD
    dloc = dst - core * SHARD
    blk = dloc // P
    slot = dloc - blk * P
    gid = core * NBLK + blk
    ngrp = N_CORES * NBLK
    # categories: 0 = lo-only (half0), 1 = flexible, 2 = hi-only (half1)
    cat = np.where(src < BASE1, 0, np.where(src < 32768, 1, 2)).astype(np.int64)

    cnt = np.bincount(gid, minlength=ngrp).reshape(N_CORES, NBLK)
    n_lo = np.bincount(gid[cat == 0], minlength=ngrp).reshape(N_CORES, NBLK)
    n_flex = np.bincount(gid[cat == 1], minlength=ngrp).reshape(N_CORES, NBLK)

    # per-block uniform K0: multiple of 128 reachable by every core
    lo = n_lo.max(axis=0)                       # [NBLK] min c0 feasible all cores
    hi = (n_lo + n_flex).min(axis=0)            # [NBLK] max c0 feasible all cores
    K0 = np.zeros(NBLK, np.int64)
    K1 = np.zeros(NBLK, np.int64)
    c0 = np.zeros((N_CORES, NBLK), np.int64)
    for b in range(NBLK):
        ks = np.arange((lo[b] + 127) // 128, hi[b] // 128 + 1)
        if len(ks) > 0:
            # feasible exact multiples: choose k minimizing total tiles,
            # tie-break toward balanced halves
            tot = ks + np.maximum(0, -(-(cnt[:, b].max() - ks * 128) // 128))
            best = ks[np.lexsort((np.abs(ks * 128 - cnt[:, b].max() // 2), tot))][0]
            K0[b] = best
            c0[:, b] = best * 128
        else:  # fallback: pad in half0 too (rare/never for these sizes)
            K0[b] = -(-lo[b] // 128)
            c0[:, b] = np.minimum(K0[b] * 128, n_lo[:, b] + n_flex[:, b])
        K1[b] = max(1, int(np.max(-(-(cnt[:, b] - c0[:, b]) // 128))))
    f0 = c0 - n_lo  # flex edges sent to half0, per (core, blk)

    # rank within (gid, cat), ordered by src for gather locality
    key_gc = gid * 3 + cat
    order1 = np.lexsort((src, key_gc))
    sk = key_gc[order1]
    starts = np.r_[0, np.flatnonzero(sk[1:] != sk[:-1]) + 1]
    start_of = np.zeros(ngrp * 3, np.int64)
    start_of[sk[starts]] = starts
    rank_gc = np.empty_like(order1)
    rank_gc[order1] = np.arange(len(order1)) - start_of[key_gc][order1]

    half = np.where(cat == 0, 0,
                    np.where(cat == 2, 1,
                             (rank_gc >= f0[core, blk]).astype(np.int64)))

    # rank within (gid, half), ordered by src
    key_gh = gid * 2 + half
    order2 = np.lexsort((src, key_gh))
    sk2 = key_gh[order2]
    starts2 = np.r_[0, np.flatnonzero(sk2[1:] != sk2[:-1]) + 1]
    start_of2 = np.zeros(ngrp * 2, np.int64)
    start_of2[sk2[starts2]] = starts2
    rank = np.empty_like(order2)
    rank[order2] = np.arange(len(order2)) - start_of2[key_gh][order2]

    # layouts (uniform): per-half tile prefixes (block-major) and
    # block-tile prefixes for slots/S
    pref0 = np.r_[0, np.cumsum(K0)]             # [NBLK+1] in tiles
    pref1 = np.r_[0, np.cumsum(K1)]
    prefT = np.r_[0, np.cumsum(K0 + K1)]
    L0 = int(pref0[-1]) * P
    L1 = int(pref1[-1]) * P
    LT = int(prefT[-1]) * P

    # Spread pad indices across the table: same-address gathers serialize
    # in the SDMA path, so don't point all pads at row 0.
    spread = ((np.arange(max(L0, L1), dtype=np.int64) * 9973) % 32768).astype(np.int16)
    idx0 = np.tile(spread[:L0], (N_CORES, 1))
    idx1 = np.tile(spread[:L1], (N_CORES, 1))
    slots = np.full((N_CORES, LT), -1.0, np.float32)

    h0 = half == 0
    h1 = ~h0
    pos0 = pref0[blk[h0]] * P + rank[h0]
    pos1 = pref1[blk[h1]] * P + rank[h1]
    idx0[core[h0], pos0] = src[h0].astype(np.int16)
    idx1[core[h1], pos1] = (src[h1] - BASE1).astype(np.int16)
    # slot positions: block-tile-major, half0 tiles then half1 tiles
    spos0 = (prefT[blk[h0]] + rank[h0] // P) * P + rank[h0] % P
    spos1 = (prefT[blk[h1]] + K0[blk[h1]] + rank[h1] // P) * P + rank[h1] % P
    slots[core[h0], spos0] = slot[h0].astype(np.float32)
    slots[core[h1], spos1] = slot[h1].astype(np.float32)

    return dict(K0=K0, K1=K1, idx0=idx0, idx1=idx1, slots=slots,
                pref0=pref0, pref1=pref1, prefT=prefT)


def _wrap_idx(idx):
    """[L] int16 -> [128, L/16] wrapped (i -> [i%16, i//16]) + replicated."""
    w = idx.reshape(-1, 16).T
    return np.ascontiguousarray(np.tile(w, (8, 1)))


def _build_program(K0, K1):
    import concourse.bacc as bacc
    import concourse.tile as tile
    import concourse.mybir as mybir
    from concourse import library_config

    f16 = mybir.dt.float16
    bf16 = mybir.dt.bfloat16
    f32 = mybir.dt.float32
    i16 = mybir.dt.int16

    K0 = list(map(int, K0))
    K1 = list(map(int, K1))
    T = [a + b for a, b in zip(K0, K1)]
    TBMAX = max(T)
    pref0 = np.r_[0, np.cumsum(K0)].astype(int)
    pref1 = np.r_[0, np.cumsum(K1)].astype(int)
    prefT = np.r_[0, np.cumsum(T)].astype(int)
    L0 = int(pref0[-1]) * P
    L1 = int(pref1[-1]) * P
    assert sum(CHUNKS) == NBLK
    chunk_starts = list(np.r_[0, np.cumsum(CHUNKS)[:-1]])

    nc = bacc.Bacc("TRN2", target_bir_lowering=False, debug=False,
                   num_devices=N_CORES, num_swdge_queues=4)
    xg_d = nc.dram_tensor("xg", [N_NODES, 128], f16, kind="ExternalInput")
    i0_d = nc.dram_tensor("i0", [128, L0 // 16], i16, kind="ExternalInput")
    i1_d = nc.dram_tensor("i1", [128, L1 // 16], i16, kind="ExternalInput")
    s_d = nc.dram_tensor("s", [P, prefT[-1]], f16, kind="ExternalInput")
    xt_d = nc.dram_tensor("xt", [IN_CH, NBLK * P], f32, kind="ExternalInput")
    wt_d = nc.dram_tensor("wt", [IN_CH, OUT_CH], bf16, kind="ExternalInput")
    b_d = nc.dram_tensor("b", [OUT_CH, 1], f32, kind="ExternalInput")
    iota_d = nc.dram_tensor("iota", [P, P], f16, kind="ExternalInput")
    wi_d = nc.dram_tensor("wi", [128, 8], i16, kind="ExternalInput")
    ident_d = nc.dram_tensor("ident", [OUT_CH, OUT_CH], f32, kind="ExternalInput")
    out_d = nc.dram_tensor("out", [SHARD, OUT_CH], f32, kind="ExternalOutput")

    with tile.TileContext(nc) as tc, ExitStack() as ctx:
        const_p = ctx.enter_context(tc.tile_pool(name="const", bufs=1))
        gat_p = ctx.enter_context(tc.tile_pool(name="gat", bufs=3))
        sel_p = ctx.enter_context(tc.tile_pool(name="sel", bufs=6))
        h_p = ctx.enter_context(tc.tile_pool(name="h", bufs=3))
        o_p = ctx.enter_context(tc.tile_pool(name="o", bufs=3))
        psum_agg = ctx.enter_context(tc.tile_pool(name="pagg", bufs=3, space="PSUM"))
        psum_mlp = ctx.enter_context(tc.tile_pool(name="pmlp", bufs=2, space="PSUM"))
        psum_tr = ctx.enter_context(tc.tile_pool(name="ptr", bufs=2, space="PSUM"))

        nc.gpsimd.load_library(library_config.mlp)

        i0_t = const_p.tile([128, L0 // 16], i16)
        i1_t = const_p.tile([128, L1 // 16], i16)
        s_t = const_p.tile([P, int(prefT[-1])], f16)
        xt_t = const_p.tile([IN_CH, NBLK * P], f32)
        wt_t = const_p.tile([IN_CH, OUT_CH], bf16)
        b_t = const_p.tile([OUT_CH, 1], f32)
        iota_t = const_p.tile([P, P], f16)
        ident_t = const_p.tile([OUT_CH, OUT_CH], f32)
        # idx tables load per-chunk on the sync queue (paces the gather
        # dispatch); other consts go via the scalar queue
        wi_t = const_p.tile([128, 8], i16)
        nc.sync.dma_start(out=wi_t[:], in_=wi_d.ap()[:])
        for t, d in [(s_t, s_d), (iota_t, iota_d), (xt_t, xt_d),
                     (wt_t, wt_d), (b_t, b_d), (ident_t, ident_d)]:
            nc.scalar.dma_start(out=t[:], in_=d.ap()[:])

        tables = [xg_d.ap()[:, :], xg_d.ap()[BASE1:, :]]
        idx_tiles = [i0_t, i1_t]
        idx_dram = [i0_d, i1_d]
        prefs = [pref0, pref1]

        qn = 0
        for ci, c0b in enumerate(chunk_starts):
            cb = CHUNKS[ci]
            t0 = [int(prefs[h][c0b]) for h in (0, 1)]
            tn = [int(prefs[h][c0b + cb]) - t0[h] for h in (0, 1)]
            for h in (0, 1):
                cA, cB_ = t0[h] * 8, (t0[h] + tn[h]) * 8
                nc.sync.dma_start(out=idx_tiles[h][:, cA:cB_],
                                  in_=idx_dram[h].ap()[:, cA:cB_])
            g = []
            for h in (0, 1):
                gt = gat_p.tile([P, tn[h], 128], f16, tag=f"g{h}",
                                name=f"g{h}_{c0b}")
                n_part = tn[h] * P
                idx_slice = idx_tiles[h][:, t0[h] * 8: t0[h] * 8 + n_part // 16]
                nc.gpsimd.dma_gather(gt[:], tables[h],
                                     idx_slice, n_part, n_part, 128,
                                     single_packet=False,
                                     queue_num=qn % 4)
                qn += 1
                g.append(gt)
            for bl in range(cb):
                blk = c0b + bl
                Tb = T[blk]
                # one-hot S for the whole block: [e, tile, slot]
                S = sel_p.tile([P, TBMAX, P], f16, name=f"S{blk}", tag="S")
                sc = int(prefT[blk])
                nc.vector.tensor_tensor(
                    out=S[:, 0:Tb, :],
                    in0=s_t[:, sc:sc + Tb][:, :, None].to_broadcast([P, Tb, P]),
                    in1=iota_t[:][:, None, :].to_broadcast([P, Tb, P]),
                    op=mybir.AluOpType.is_equal,
                )
                pa = psum_agg.tile([IN_CH, P], f32, space="PSUM")
                for j in range(Tb):
                    if j < K0[blk]:
                        gh, gidx = 0, (int(pref0[blk]) - t0[0]) + j
                    else:
                        gh, gidx = 1, (int(pref1[blk]) - t0[1]) + (j - K0[blk])
                    nc.tensor.matmul(
                        out=pa[:],
                        lhsT=g[gh][:, gidx, :IN_CH],
                        rhs=S[:, j, :],
                        start=(j == 0),
                        stop=(j == Tb - 1),
                    )
                h_t = h_p.tile([IN_CH, P], bf16)
                nc.vector.tensor_add(out=h_t[:], in0=pa[:],
                                     in1=xt_t[:, blk * P:(blk + 1) * P])
                pm = psum_mlp.tile([OUT_CH, P], f32, space="PSUM")
                nc.tensor.matmul(out=pm[:], lhsT=wt_t[:], rhs=h_t[:],
                                 start=True, stop=True)
                r_t = h_p.tile([OUT_CH, P], f32, tag="r")
                nc.scalar.activation(out=r_t[:], in_=pm[:],
                                     func=mybir.ActivationFunctionType.Relu,
                                     bias=b_t[:])
                pt = psum_tr.tile([P, OUT_CH], f32, space="PSUM")
                nc.tensor.transpose(out=pt[:], in_=r_t[:], identity=ident_t[:])
                rows = min(P, SHARD - blk * P)
                o_t = o_p.tile([P, OUT_CH], f32)
                nc.scalar.activation(out=o_t[:], in_=pt[:],
                                     func=mybir.ActivationFunctionType.Copy)
                nc.sync.dma_start(out=out_d.ap()[blk * P: blk * P + rows, :],
                                  in_=o_t[:rows, :])

    nc.compile()
    return nc


def _prepare(x, edge_index, W, b):
    """Host-side routing + per-core input maps. Returns (in_maps, route)."""
    f16np = np.float16
    x = np.asarray(x, np.float32)
    W = np.asarray(W, np.float32)
    b = np.asarray(b, np.float32)
    src = np.asarray(edge_index[0])
    dst = np.asarray(edge_index[1])

    r = _route(src, dst)
    TBMAX = int((r["K0"] + r["K1"]).max())

    xg = np.zeros((N_NODES, 128), f16np)
    xg[:, :IN_CH] = x.astype(f16np)
    iota = np.tile(np.arange(P, dtype=np.float32), (P, 1)).astype(f16np)
    ident = np.eye(OUT_CH, dtype=np.float32)
    wt = np.ascontiguousarray(W.T).astype(ml_dtypes.bfloat16)
    b2 = np.ascontiguousarray(b.reshape(-1, 1))

    in_maps = []
    for c in range(N_CORES):
        xt = np.zeros((IN_CH, NBLK * P), np.float32)
        xt[:, :SHARD] = x[c * SHARD:(c + 1) * SHARD].T
        slots = r["slots"][c]
        in_maps.append({
            "xg": xg,
            "wi": np.ascontiguousarray(
                ((np.arange(128, dtype=np.int64) * 9973) % 32768
                 ).astype(np.int16).reshape(-1, 16).T.repeat(8, axis=0)
                ).reshape(128, 8),
            "i0": _wrap_idx(r["idx0"][c]),
            "i1": _wrap_idx(r["idx1"][c]),
            "s": np.ascontiguousarray(slots.reshape(-1, P).T).astype(f16np),
            "xt": np.ascontiguousarray(xt),
            "wt": wt,
            "b": b2,
            "iota": iota,
            "ident": ident,
        })
    return in_maps, r


_CACHE = {}


def _get_program(K0, K1):
    key = (tuple(K0), tuple(K1))
    if key not in _CACHE:
        _CACHE[key] = _build_program(K0, K1)
    return _CACHE[key]


def _best_effort_device_reset():
    """If a previous process wedged the NeuronCores, a reset lets this
    process's run succeed. Harmless (rc=0, state-free) on a healthy device."""
    try:
        import ctypes, jax
        jax.devices()
        lib = ctypes.CDLL("/opt/axon/libaxon_pjrt.so")
        lib.axon_reset.restype = ctypes.c_int64
        lib.axon_reset()
    except Exception:
        pass


def run(x, edge_index, W, b, trace=False):
    from concourse.bass_utils import run_bass_kernel_spmd
    _best_effort_device_reset()
    in_maps, r = _prepare(x, edge_index, W, b)
    nc = _get_program(r["K0"], r["K1"])
    res = run_bass_kernel_spmd(nc, in_maps, core_ids=list(range(N_CORES)),
                               trace=trace)
    out = np.concatenate([res.results[c]["out"] for c in range(N_CORES)], axis=0)
    return out.astype(np.float32), res


def kernel(x, edge_index, W, b):
    out, _ = run(x, edge_index, W, b, trace=False)
    return out


# revision 58
# speedup vs baseline: 1.0672x; 1.0672x over previous
"""DirectionalGINConv (eps=0) Trainium2 kernel, 8-core SPMD.

  agg_i = sum_{j->i} x_j ; out = relu(relu((x + agg) @ W.T + b))

Strategy (hardcoded for N=50000, E=800000, C=64, 8 cores):
  - Destination-node sharding: core c owns dst rows [c*6250, (c+1)*6250).
  - Host routes edges into per-(dst-block-of-128) tile groups of 128 edges.
    Each block b gets T_b gather tiles (uniform across cores for SPMD):
    the first K0_b tiles hold edges with src < 32768 (gather table base row
    0), the rest hold src >= 17232 (base row 17232), so gather indices fit
    in int16 (dma_gather limit). K0_b*128 is chosen so every core can route
    exactly K0_b*128 low-src edges to block b (zero pad in half0); only the
    half1 tail tile of each block carries pad slots.
  - Device per core: dma_gather x rows (fp16, rows padded to 128ch = 256B)
    block-grouped; two gather calls per chunk of blocks, round-robinned
    over all 4 SWDGE queues with all index tiles preloaded, so ~4 gather
    descriptor streams stay in flight (the SWDGE desc-gen CPU pair per
    queue is the bottleneck, not the DMA engines).
  - Segment-sum via PE: per block build one-hot S[e, slot, j] on DVE with
    packed last dims (2x DVE mode) in a single op, then T_b accumulating
    matmuls psum[ch, slot] += G_tile.T @ S[:, :, j]; h = psum + x_shard.T;
    MLP = W.T-stationary matmul; relu+bias on ACT; PE transpose back to
    node-major; DMA out.
"""

import numpy as np
from contextlib import ExitStack

import ml_dtypes

N_NODES = 50000
IN_CH = 64
OUT_CH = 64
N_CORES = 8
SHARD = N_NODES // N_CORES          # 6250
P = 128
NBLK = (SHARD + P - 1) // P         # 49 blocks (last has 106 slots)
BASE1 = 17232                       # half1 table base (50000 - 32768)
CHUNKS = [5] * 9 + [2, 2]           # blocks per gather chunk (sum=49); each
                                    # (chunk, half) gather splits into 2
                                    # sub-calls (~2.5-3k indices each: the
                                    # SWDGE desc-gen slows beyond ~3.1k/call)


def _route(src, dst):
    """Vectorized edge routing with uniform-across-cores variable tiling.

    Returns dict with:
      K0, K1: [NBLK] int arrays (tiles per half, uniform across cores)
      idx0, idx1: [N_CORES, L0], [N_CORES, L1] int16 gather indices
      slots: [N_CORES, LT] float32 slot-in-block (-1 pad), block-tile-major
    where L0 = sum(K0)*128, L1 = sum(K1)*128, LT = L0+L1.
    """
    src = np.asarray(src, np.int64)
    dst = np.asarray(dst, np.int64)
    core = dst // SHARD
    dloc = dst - core * SHARD
    blk = dloc // P
    slot = dloc - blk * P
    gid = core * NBLK + blk
    ngrp = N_CORES * NBLK
    # categories: 0 = lo-only (half0), 1 = flexible, 2 = hi-only (half1)
    cat = np.where(src < BASE1, 0, np.where(src < 32768, 1, 2)).astype(np.int64)

    cnt = np.bincount(gid, minlength=ngrp).reshape(N_CORES, NBLK)
    n_lo = np.bincount(gid[cat == 0], minlength=ngrp).reshape(N_CORES, NBLK)
    n_flex = np.bincount(gid[cat == 1], minlength=ngrp).reshape(N_CORES, NBLK)

    # per-block uniform K0: multiple of 128 reachable by every core
    lo = n_lo.max(axis=0)                       # [NBLK] min c0 feasible all cores
    hi = (n_lo + n_flex).min(axis=0)            # [NBLK] max c0 feasible all cores
    K0 = np.zeros(NBLK, np.int64)
    K1 = np.zeros(NBLK, np.int64)
    c0 = np.zeros((N_CORES, NBLK), np.int64)
    for b in range(NBLK):
        ks = np.arange((lo[b] + 127) // 128, hi[b] // 128 + 1)
        if len(ks) > 0:
            # feasible exact multiples: choose k minimizing total tiles,
            # tie-break toward balanced halves
            tot = ks + np.maximum(0, -(-(cnt[:, b].max() - ks * 128) // 128))
            best = ks[np.lexsort((np.abs(ks * 128 - cnt[:, b].max() // 2), tot))][0]
            K0[b] = best
            c0[:, b] = best * 128
        else:  # fallback: pad in half0 too (rare/never for these sizes)
            K0[b] = -(-lo[b] // 128)
            c0[:, b] = np.minimum(K0[b] * 128, n_lo[:, b] + n_flex[:, b])
        K1[b] = max(1, int(np.max(-(-(cnt[:, b] - c0[:, b]) // 128))))
    f0 = c0 - n_lo  # flex edges sent to half0, per (core, blk)

    # rank within (gid, cat), ordered by src for gather locality
    key_gc = gid * 3 + cat
    order1 = np.lexsort((src, key_gc))
    sk = key_gc[order1]
    starts = np.r_[0, np.flatnonzero(sk[1:] != sk[:-1]) + 1]
    start_of = np.zeros(ngrp * 3, np.int64)
    start_of[sk[starts]] = starts
    rank_gc = np.empty_like(order1)
    rank_gc[order1] = np.arange(len(order1)) - start_of[key_gc][order1]

    half = np.where(cat == 0, 0,
                    np.where(cat == 2, 1,
                             (rank_gc >= f0[core, blk]).astype(np.int64)))

    # rank within (gid, half), ordered by src
    key_gh = gid * 2 + half
    order2 = np.lexsort((src, key_gh))
    sk2 = key_gh[order2]
    starts2 = np.r_[0, np.flatnonzero(sk2[1:] != sk2[:-1]) + 1]
    start_of2 = np.zeros(ngrp * 2, np.int64)
    start_of2[sk2[starts2]] = starts2
    rank = np.empty_like(order2)
    rank[order2] = np.arange(len(order2)) - start_of2[key_gh][order2]

    # layouts (uniform): per-half tile prefixes (block-major) and
    # block-tile prefixes for slots/S
    pref0 = np.r_[0, np.cumsum(K0)]             # [NBLK+1] in tiles
    pref1 = np.r_[0, np.cumsum(K1)]
    prefT = np.r_[0, np.cumsum(K0 + K1)]
    L0 = int(pref0[-1]) * P
    L1 = int(pref1[-1]) * P
    LT = int(prefT[-1]) * P

    # Spread pad indices across the table: same-address gathers serialize
    # in the SDMA path, so don't point all pads at row 0.
    spread = ((np.arange(max(L0, L1), dtype=np.int64) * 9973) % 32768).astype(np.int16)
    idx0 = np.tile(spread[:L0], (N_CORES, 1))
    idx1 = np.tile(spread[:L1], (N_CORES, 1))
    slots = np.full((N_CORES, LT), -1.0, np.float32)

    h0 = half == 0
    h1 = ~h0
    pos0 = pref0[blk[h0]] * P + rank[h0]
    pos1 = pref1[blk[h1]] * P + rank[h1]
    idx0[core[h0], pos0] = src[h0].astype(np.int16)
    idx1[core[h1], pos1] = (src[h1] - BASE1).astype(np.int16)
    # slot positions: block-tile-major, half0 tiles then half1 tiles
    spos0 = (prefT[blk[h0]] + rank[h0] // P) * P + rank[h0] % P
    spos1 = (prefT[blk[h1]] + K0[blk[h1]] + rank[h1] // P) * P + rank[h1] % P
    slots[core[h0], spos0] = slot[h0].astype(np.float32)
    slots[core[h1], spos1] = slot[h1].astype(np.float32)

    return dict(K0=K0, K1=K1, idx0=idx0, idx1=idx1, slots=slots,
                pref0=pref0, pref1=pref1, prefT=prefT)


def _wrap_idx(idx):
    """[L] int16 -> [128, L/16] wrapped (i -> [i%16, i//16]) + replicated."""
    w = idx.reshape(-1, 16).T
    return np.ascontiguousarray(np.tile(w, (8, 1)))


def _build_program(K0, K1):
    import concourse.bacc as bacc
    import concourse.tile as tile
    import concourse.mybir as mybir
    from concourse import library_config

    f16 = mybir.dt.float16
    bf16 = mybir.dt.bfloat16
    f32 = mybir.dt.float32
    i16 = mybir.dt.int16

    K0 = list(map(int, K0))
    K1 = list(map(int, K1))
    T = [a + b for a, b in zip(K0, K1)]
    TBMAX = max(T)
    pref0 = np.r_[0, np.cumsum(K0)].astype(int)
    pref1 = np.r_[0, np.cumsum(K1)].astype(int)
    prefT = np.r_[0, np.cumsum(T)].astype(int)
    L0 = int(pref0[-1]) * P
    L1 = int(pref1[-1]) * P
    assert sum(CHUNKS) == NBLK
    chunk_starts = list(np.r_[0, np.cumsum(CHUNKS)[:-1]])

    nc = bacc.Bacc("TRN2", target_bir_lowering=False, debug=False,
                   num_devices=N_CORES, num_swdge_queues=4)
    xg_d = nc.dram_tensor("xg", [N_NODES, 128], f16, kind="ExternalInput")
    i0_d = nc.dram_tensor("i0", [128, L0 // 16], i16, kind="ExternalInput")
    i1_d = nc.dram_tensor("i1", [128, L1 // 16], i16, kind="ExternalInput")
    s_d = nc.dram_tensor("s", [P, prefT[-1]], f16, kind="ExternalInput")
    xt_d = nc.dram_tensor("xt", [IN_CH, NBLK * P], f32, kind="ExternalInput")
    wt_d = nc.dram_tensor("wt", [IN_CH, OUT_CH], bf16, kind="ExternalInput")
    b_d = nc.dram_tensor("b", [OUT_CH, 1], f32, kind="ExternalInput")
    iota_d = nc.dram_tensor("iota", [P, P], f16, kind="ExternalInput")
    wi_d = nc.dram_tensor("wi", [128, 8], i16, kind="ExternalInput")
    ident_d = nc.dram_tensor("ident", [OUT_CH, OUT_CH], f32, kind="ExternalInput")
    out_d = nc.dram_tensor("out", [SHARD, OUT_CH], f32, kind="ExternalOutput")

    with tile.TileContext(nc) as tc, ExitStack() as ctx:
        const_p = ctx.enter_context(tc.tile_pool(name="const", bufs=1))
        gat_p = ctx.enter_context(tc.tile_pool(name="gat", bufs=3))
        sel_p = ctx.enter_context(tc.tile_pool(name="sel", bufs=6))
        h_p = ctx.enter_context(tc.tile_pool(name="h", bufs=3))
        o_p = ctx.enter_context(tc.tile_pool(name="o", bufs=3))
        psum_agg = ctx.enter_context(tc.tile_pool(name="pagg", bufs=3, space="PSUM"))
        psum_mlp = ctx.enter_context(tc.tile_pool(name="pmlp", bufs=2, space="PSUM"))
        psum_tr = ctx.enter_context(tc.tile_pool(name="ptr", bufs=2, space="PSUM"))

        nc.gpsimd.load_library(library_config.mlp)

        i0_t = const_p.tile([128, L0 // 16], i16)
        i1_t = const_p.tile([128, L1 // 16], i16)
        s_t = const_p.tile([P, int(prefT[-1])], f16)
        xt_t = const_p.tile([IN_CH, NBLK * P], f32)
        wt_t = const_p.tile([IN_CH, OUT_CH], bf16)
        b_t = const_p.tile([OUT_CH, 1], f32)
        iota_t = const_p.tile([P, P], f16)
        ident_t = const_p.tile([OUT_CH, OUT_CH], f32)
        # idx tables load per-chunk on the sync queue (paces the gather
        # dispatch); other consts go via the scalar queue
        wi_t = const_p.tile([128, 8], i16)
        nc.sync.dma_start(out=wi_t[:], in_=wi_d.ap()[:])
        for t, d in [(s_t, s_d), (iota_t, iota_d), (xt_t, xt_d),
                     (wt_t, wt_d), (b_t, b_d), (ident_t, ident_d)]:
            nc.scalar.dma_start(out=t[:], in_=d.ap()[:])

        tables = [xg_d.ap()[:, :], xg_d.ap()[BASE1:, :]]
        idx_tiles = [i0_t, i1_t]
        idx_dram = [i0_d, i1_d]
        prefs = [pref0, pref1]

        qn = 0
        for ci, c0b in enumerate(chunk_starts):
            cb = CHUNKS[ci]
            t0 = [int(prefs[h][c0b]) for h in (0, 1)]
            tn = [int(prefs[h][c0b + cb]) - t0[h] for h in (0, 1)]
            for h in (0, 1):
                cA, cB_ = t0[h] * 8, (t0[h] + tn[h]) * 8
                nc.sync.dma_start(out=idx_tiles[h][:, cA:cB_],
                                  in_=idx_dram[h].ap()[:, cA:cB_])
            g = []
            for h in (0, 1):
                gt = gat_p.tile([P, tn[h], 128], f16, tag=f"g{h}",
                                name=f"g{h}_{c0b}")
                p0 = tn[h] // 2
                for off, cnt in ((0, p0), (p0, tn[h] - p0)):
                    if cnt <= 0:
                        continue
                    n_part = cnt * P
                    col0 = (t0[h] + off) * 8
                    idx_slice = idx_tiles[h][:, col0: col0 + n_part // 16]
                    nc.gpsimd.dma_gather(gt[:, off:off + cnt, :], tables[h],
                                         idx_slice, n_part, n_part, 128,
                                         single_packet=False,
                                         queue_num=qn % 4)
                    qn += 1
                g.append(gt)
            for bl in range(cb):
                blk = c0b + bl
                Tb = T[blk]
                # one-hot S for the whole block: [e, tile, slot]
                S = sel_p.tile([P, TBMAX, P], f16, name=f"S{blk}", tag="S")
                sc = int(prefT[blk])
                nc.vector.tensor_tensor(
                    out=S[:, 0:Tb, :],
                    in0=s_t[:, sc:sc + Tb][:, :, None].to_broadcast([P, Tb, P]),
                    in1=iota_t[:][:, None, :].to_broadcast([P, Tb, P]),
                    op=mybir.AluOpType.is_equal,
                )
                pa = psum_agg.tile([IN_CH, P], f32, space="PSUM")
                for j in range(Tb):
                    if j < K0[blk]:
                        gh, gidx = 0, (int(pref0[blk]) - t0[0]) + j
                    else:
                        gh, gidx = 1, (int(pref1[blk]) - t0[1]) + (j - K0[blk])
                    nc.tensor.matmul(
                        out=pa[:],
                        lhsT=g[gh][:, gidx, :IN_CH],
                        rhs=S[:, j, :],
                        start=(j == 0),
                        stop=(j == Tb - 1),
                    )
                h_t = h_p.tile([IN_CH, P], bf16)
                nc.vector.tensor_add(out=h_t[:], in0=pa[:],
                                     in1=xt_t[:, blk * P:(blk + 1) * P])
                pm = psum_mlp.tile([OUT_CH, P], f32, space="PSUM")
                nc.tensor.matmul(out=pm[:], lhsT=wt_t[:], rhs=h_t[:],
                                 start=True, stop=True)
                r_t = h_p.tile([OUT_CH, P], f32, tag="r")
                nc.scalar.activation(out=r_t[:], in_=pm[:],
                                     func=mybir.ActivationFunctionType.Relu,
                                     bias=b_t[:])
                pt = psum_tr.tile([P, OUT_CH], f32, space="PSUM")
                nc.tensor.transpose(out=pt[:], in_=r_t[:], identity=ident_t[:])
                rows = min(P, SHARD - blk * P)
                o_t = o_p.tile([P, OUT_CH], f32)
                nc.scalar.activation(out=o_t[:], in_=pt[:],
                                     func=mybir.ActivationFunctionType.Copy)
                nc.sync.dma_start(out=out_d.ap()[blk * P: blk * P + rows, :],
                                  in_=o_t[:rows, :])

    nc.compile()
    return nc


def _prepare(x, edge_index, W, b):
    """Host-side routing + per-core input maps. Returns (in_maps, route)."""
    f16np = np.float16
    x = np.asarray(x, np.float32)
    W = np.asarray(W, np.float32)
    b = np.asarray(b, np.float32)
    src = np.asarray(edge_index[0])
    dst = np.asarray(edge_index[1])

    r = _route(src, dst)
    TBMAX = int((r["K0"] + r["K1"]).max())

    xg = np.zeros((N_NODES, 128), f16np)
    xg[:, :IN_CH] = x.astype(f16np)
    iota = np.tile(np.arange(P, dtype=np.float32), (P, 1)).astype(f16np)
    ident = np.eye(OUT_CH, dtype=np.float32)
    wt = np.ascontiguousarray(W.T).astype(ml_dtypes.bfloat16)
    b2 = np.ascontiguousarray(b.reshape(-1, 1))

    in_maps = []
    for c in range(N_CORES):
        xt = np.zeros((IN_CH, NBLK * P), np.float32)
        xt[:, :SHARD] = x[c * SHARD:(c + 1) * SHARD].T
        slots = r["slots"][c]
        in_maps.append({
            "xg": xg,
            "wi": np.ascontiguousarray(
                ((np.arange(128, dtype=np.int64) * 9973) % 32768
                 ).astype(np.int16).reshape(-1, 16).T.repeat(8, axis=0)
                ).reshape(128, 8),
            "i0": _wrap_idx(r["idx0"][c]),
            "i1": _wrap_idx(r["idx1"][c]),
            "s": np.ascontiguousarray(slots.reshape(-1, P).T).astype(f16np),
            "xt": np.ascontiguousarray(xt),
            "wt": wt,
            "b": b2,
            "iota": iota,
            "ident": ident,
        })
    return in_maps, r


_CACHE = {}


def _get_program(K0, K1):
    key = (tuple(K0), tuple(K1))
    if key not in _CACHE:
        _CACHE[key] = _build_program(K0, K1)
    return _CACHE[key]


def _best_effort_device_reset():
    """If a previous process wedged the NeuronCores, a reset lets this
    process's run succeed. Harmless (rc=0, state-free) on a healthy device."""
    try:
        import ctypes, jax
        jax.devices()
        lib = ctypes.CDLL("/opt/axon/libaxon_pjrt.so")
        lib.axon_reset.restype = ctypes.c_int64
        lib.axon_reset()
    except Exception:
        pass


def run(x, edge_index, W, b, trace=False):
    from concourse.bass_utils import run_bass_kernel_spmd
    _best_effort_device_reset()
    in_maps, r = _prepare(x, edge_index, W, b)
    nc = _get_program(r["K0"], r["K1"])
    res = run_bass_kernel_spmd(nc, in_maps, core_ids=list(range(N_CORES)),
                               trace=trace)
    out = np.concatenate([res.results[c]["out"] for c in range(N_CORES)], axis=0)
    return out.astype(np.float32), res


def kernel(x, edge_index, W, b):
    out, _ = run(x, edge_index, W, b, trace=False)
    return out


# revision 61
# speedup vs baseline: 1.0825x; 1.0143x over previous
"""DirectionalGINConv (eps=0) Trainium2 kernel, 8-core SPMD.

  agg_i = sum_{j->i} x_j ; out = relu(relu((x + agg) @ W.T + b))

Strategy (hardcoded for N=50000, E=800000, C=64, 8 cores):
  - Destination-node sharding: core c owns dst rows [c*6250, (c+1)*6250).
  - Host routes edges into per-(dst-block-of-128) tile groups of 128 edges.
    Each block b gets T_b gather tiles (uniform across cores for SPMD):
    the first K0_b tiles hold edges with src < 32768 (gather table base row
    0), the rest hold src >= 17232 (base row 17232), so gather indices fit
    in int16 (dma_gather limit). K0_b*128 is chosen so every core can route
    exactly K0_b*128 low-src edges to block b (zero pad in half0); only the
    half1 tail tile of each block carries pad slots.
  - Device per core: dma_gather x rows (fp16, rows padded to 128ch = 256B)
    block-grouped; two gather calls per chunk of blocks, round-robinned
    over all 4 SWDGE queues with all index tiles preloaded, so ~4 gather
    descriptor streams stay in flight (the SWDGE desc-gen CPU pair per
    queue is the bottleneck, not the DMA engines).
  - Segment-sum via PE: per block build one-hot S[e, slot, j] on DVE with
    packed last dims (2x DVE mode) in a single op, then T_b accumulating
    matmuls psum[ch, slot] += G_tile.T @ S[:, :, j]; h = psum + x_shard.T;
    MLP = W.T-stationary matmul; relu+bias on ACT; PE transpose back to
    node-major; DMA out.
"""

import numpy as np
from contextlib import ExitStack

import ml_dtypes

N_NODES = 50000
IN_CH = 64
OUT_CH = 64
N_CORES = 8
SHARD = N_NODES // N_CORES          # 6250
P = 128
NBLK = (SHARD + P - 1) // P         # 49 blocks (last has 106 slots)
BASE1 = 17232                       # half1 table base (50000 - 32768)
CHUNKS = [5] * 9 + [2, 2]           # blocks per gather chunk (sum=49); each
                                    # (chunk, half) gather splits into 2
                                    # sub-calls (~2.5-3k indices each: the
                                    # SWDGE desc-gen slows beyond ~3.1k/call)


def _route(src, dst):
    """Vectorized edge routing with uniform-across-cores variable tiling.

    Returns dict with:
      K0, K1: [NBLK] int arrays (tiles per half, uniform across cores)
      idx0, idx1: [N_CORES, L0], [N_CORES, L1] int16 gather indices
      slots: [N_CORES, LT] float32 slot-in-block (-1 pad), block-tile-major
    where L0 = sum(K0)*128, L1 = sum(K1)*128, LT = L0+L1.
    """
    src = np.asarray(src, np.int64)
    dst = np.asarray(dst, np.int64)
    core = dst // SHARD
    dloc = dst - core * SHARD
    blk = dloc // P
    slot = dloc - blk * P
    gid = core * NBLK + blk
    ngrp = N_CORES * NBLK
    # categories: 0 = lo-only (half0), 1 = flexible, 2 = hi-only (half1)
    cat = np.where(src < BASE1, 0, np.where(src < 32768, 1, 2)).astype(np.int64)

    cnt = np.bincount(gid, minlength=ngrp).reshape(N_CORES, NBLK)
    n_lo = np.bincount(gid[cat == 0], minlength=ngrp).reshape(N_CORES, NBLK)
    n_flex = np.bincount(gid[cat == 1], minlength=ngrp).reshape(N_CORES, NBLK)

    # per-block uniform K0: multiple of 128 reachable by every core
    lo = n_lo.max(axis=0)                       # [NBLK] min c0 feasible all cores
    hi = (n_lo + n_flex).min(axis=0)            # [NBLK] max c0 feasible all cores
    K0 = np.zeros(NBLK, np.int64)
    K1 = np.zeros(NBLK, np.int64)
    c0 = np.zeros((N_CORES, NBLK), np.int64)
    for b in range(NBLK):
        ks = np.arange((lo[b] + 127) // 128, hi[b] // 128 + 1)
        if len(ks) > 0:
            # feasible exact multiples: choose k minimizing total tiles,
            # tie-break toward balanced halves
            tot = ks + np.maximum(0, -(-(cnt[:, b].max() - ks * 128) // 128))
            best = ks[np.lexsort((np.abs(ks * 128 - cnt[:, b].max() // 2), tot))][0]
            K0[b] = best
            c0[:, b] = best * 128
        else:  # fallback: pad in half0 too (rare/never for these sizes)
            K0[b] = -(-lo[b] // 128)
            c0[:, b] = np.minimum(K0[b] * 128, n_lo[:, b] + n_flex[:, b])
        K1[b] = max(1, int(np.max(-(-(cnt[:, b] - c0[:, b]) // 128))))
    f0 = c0 - n_lo  # flex edges sent to half0, per (core, blk)

    # rank within (gid, cat), ordered by src for gather locality
    key_gc = gid * 3 + cat
    order1 = np.lexsort((src, key_gc))
    sk = key_gc[order1]
    starts = np.r_[0, np.flatnonzero(sk[1:] != sk[:-1]) + 1]
    start_of = np.zeros(ngrp * 3, np.int64)
    start_of[sk[starts]] = starts
    rank_gc = np.empty_like(order1)
    rank_gc[order1] = np.arange(len(order1)) - start_of[key_gc][order1]

    half = np.where(cat == 0, 0,
                    np.where(cat == 2, 1,
                             (rank_gc >= f0[core, blk]).astype(np.int64)))

    # rank within (gid, half), ordered by src
    key_gh = gid * 2 + half
    order2 = np.lexsort((src, key_gh))
    sk2 = key_gh[order2]
    starts2 = np.r_[0, np.flatnonzero(sk2[1:] != sk2[:-1]) + 1]
    start_of2 = np.zeros(ngrp * 2, np.int64)
    start_of2[sk2[starts2]] = starts2
    rank = np.empty_like(order2)
    rank[order2] = np.arange(len(order2)) - start_of2[key_gh][order2]

    # layouts (uniform): per-half tile prefixes (block-major) and
    # block-tile prefixes for slots/S
    pref0 = np.r_[0, np.cumsum(K0)]             # [NBLK+1] in tiles
    pref1 = np.r_[0, np.cumsum(K1)]
    prefT = np.r_[0, np.cumsum(K0 + K1)]
    L0 = int(pref0[-1]) * P
    L1 = int(pref1[-1]) * P
    LT = int(prefT[-1]) * P

    # Spread pad indices across the table: same-address gathers serialize
    # in the SDMA path, so don't point all pads at row 0.
    spread = ((np.arange(max(L0, L1), dtype=np.int64) * 9973) % 32768).astype(np.int16)
    idx0 = np.tile(spread[:L0], (N_CORES, 1))
    idx1 = np.tile(spread[:L1], (N_CORES, 1))
    slots = np.full((N_CORES, LT), -1.0, np.float32)

    h0 = half == 0
    h1 = ~h0
    pos0 = pref0[blk[h0]] * P + rank[h0]
    pos1 = pref1[blk[h1]] * P + rank[h1]
    idx0[core[h0], pos0] = src[h0].astype(np.int16)
    idx1[core[h1], pos1] = (src[h1] - BASE1).astype(np.int16)
    # slot positions: block-tile-major, half0 tiles then half1 tiles
    spos0 = (prefT[blk[h0]] + rank[h0] // P) * P + rank[h0] % P
    spos1 = (prefT[blk[h1]] + K0[blk[h1]] + rank[h1] // P) * P + rank[h1] % P
    slots[core[h0], spos0] = slot[h0].astype(np.float32)
    slots[core[h1], spos1] = slot[h1].astype(np.float32)

    return dict(K0=K0, K1=K1, idx0=idx0, idx1=idx1, slots=slots,
                pref0=pref0, pref1=pref1, prefT=prefT)


def _wrap_idx(idx):
    """[L] int16 -> [128, L/16] wrapped (i -> [i%16, i//16]) + replicated."""
    w = idx.reshape(-1, 16).T
    return np.ascontiguousarray(np.tile(w, (8, 1)))


def _build_program(K0, K1):
    import concourse.bacc as bacc
    import concourse.tile as tile
    import concourse.mybir as mybir
    from concourse import library_config

    f16 = mybir.dt.float16
    bf16 = mybir.dt.bfloat16
    f32 = mybir.dt.float32
    i16 = mybir.dt.int16

    K0 = list(map(int, K0))
    K1 = list(map(int, K1))
    T = [a + b for a, b in zip(K0, K1)]
    TBMAX = max(T)
    pref0 = np.r_[0, np.cumsum(K0)].astype(int)
    pref1 = np.r_[0, np.cumsum(K1)].astype(int)
    prefT = np.r_[0, np.cumsum(T)].astype(int)
    L0 = int(pref0[-1]) * P
    L1 = int(pref1[-1]) * P
    assert sum(CHUNKS) == NBLK
    chunk_starts = list(np.r_[0, np.cumsum(CHUNKS)[:-1]])

    nc = bacc.Bacc("TRN2", target_bir_lowering=False, debug=False,
                   num_devices=N_CORES, num_swdge_queues=4)
    xg_d = nc.dram_tensor("xg", [N_NODES, 128], f16, kind="ExternalInput")
    i0_d = nc.dram_tensor("i0", [128, L0 // 16], i16, kind="ExternalInput")
    i1_d = nc.dram_tensor("i1", [128, L1 // 16], i16, kind="ExternalInput")
    s_d = nc.dram_tensor("s", [P, prefT[-1]], f16, kind="ExternalInput")
    xt_d = nc.dram_tensor("xt", [IN_CH, NBLK * P], f32, kind="ExternalInput")
    wt_d = nc.dram_tensor("wt", [IN_CH, OUT_CH], bf16, kind="ExternalInput")
    b_d = nc.dram_tensor("b", [OUT_CH, 1], f32, kind="ExternalInput")
    iota_d = nc.dram_tensor("iota", [P, P], f16, kind="ExternalInput")
    ident_d = nc.dram_tensor("ident", [OUT_CH, OUT_CH], f32, kind="ExternalInput")
    out_d = nc.dram_tensor("out", [SHARD, OUT_CH], f32, kind="ExternalOutput")

    with tile.TileContext(nc) as tc, ExitStack() as ctx:
        const_p = ctx.enter_context(tc.tile_pool(name="const", bufs=1))
        gat_p = ctx.enter_context(tc.tile_pool(name="gat", bufs=3))
        sel_p = ctx.enter_context(tc.tile_pool(name="sel", bufs=6))
        h_p = ctx.enter_context(tc.tile_pool(name="h", bufs=3))
        o_p = ctx.enter_context(tc.tile_pool(name="o", bufs=3))
        psum_agg = ctx.enter_context(tc.tile_pool(name="pagg", bufs=3, space="PSUM"))
        psum_mlp = ctx.enter_context(tc.tile_pool(name="pmlp", bufs=2, space="PSUM"))
        psum_tr = ctx.enter_context(tc.tile_pool(name="ptr", bufs=2, space="PSUM"))

        nc.gpsimd.load_library(library_config.mlp)

        i0_t = const_p.tile([128, L0 // 16], i16)
        i1_t = const_p.tile([128, L1 // 16], i16)
        s_t = const_p.tile([P, int(prefT[-1])], f16)
        xt_t = const_p.tile([IN_CH, NBLK * P], f32)
        wt_t = const_p.tile([IN_CH, OUT_CH], bf16)
        b_t = const_p.tile([OUT_CH, 1], f32)
        iota_t = const_p.tile([P, P], f16)
        ident_t = const_p.tile([OUT_CH, OUT_CH], f32)
        # idx tables load per-chunk on the sync queue (paces the gather
        # dispatch); other consts go via the scalar queue, the big xt last
        # so it can't gate anything early
        for t, d in [(s_t, s_d), (iota_t, iota_d), (wt_t, wt_d),
                     (b_t, b_d), (ident_t, ident_d), (xt_t, xt_d)]:
            nc.scalar.dma_start(out=t[:], in_=d.ap()[:])

        tables = [xg_d.ap()[:, :], xg_d.ap()[BASE1:, :]]
        idx_tiles = [i0_t, i1_t]
        idx_dram = [i0_d, i1_d]
        prefs = [pref0, pref1]

        qn = 0
        for ci, c0b in enumerate(chunk_starts):
            cb = CHUNKS[ci]
            t0 = [int(prefs[h][c0b]) for h in (0, 1)]
            tn = [int(prefs[h][c0b + cb]) - t0[h] for h in (0, 1)]
            for h in (0, 1):
                cA, cB_ = t0[h] * 8, (t0[h] + tn[h]) * 8
                nc.sync.dma_start(out=idx_tiles[h][:, cA:cB_],
                                  in_=idx_dram[h].ap()[:, cA:cB_])
            g = []
            for h in (0, 1):
                gt = gat_p.tile([P, tn[h], 128], f16, tag=f"g{h}",
                                name=f"g{h}_{c0b}")
                p0 = tn[h] // 2
                for off, cnt in ((0, p0), (p0, tn[h] - p0)):
                    if cnt <= 0:
                        continue
                    n_part = cnt * P
                    col0 = (t0[h] + off) * 8
                    idx_slice = idx_tiles[h][:, col0: col0 + n_part // 16]
                    nc.gpsimd.dma_gather(gt[:, off:off + cnt, :], tables[h],
                                         idx_slice, n_part, n_part, 128,
                                         single_packet=False,
                                         queue_num=qn % 4)
                    qn += 1
                g.append(gt)
            for bl in range(cb):
                blk = c0b + bl
                Tb = T[blk]
                # one-hot S for the whole block: [e, tile, slot]
                S = sel_p.tile([P, TBMAX, P], f16, name=f"S{blk}", tag="S")
                sc = int(prefT[blk])
                nc.vector.tensor_tensor(
                    out=S[:, 0:Tb, :],
                    in0=s_t[:, sc:sc + Tb][:, :, None].to_broadcast([P, Tb, P]),
                    in1=iota_t[:][:, None, :].to_broadcast([P, Tb, P]),
                    op=mybir.AluOpType.is_equal,
                )
                pa = psum_agg.tile([IN_CH, P], f32, space="PSUM")
                for j in range(Tb):
                    if j < K0[blk]:
                        gh, gidx = 0, (int(pref0[blk]) - t0[0]) + j
                    else:
                        gh, gidx = 1, (int(pref1[blk]) - t0[1]) + (j - K0[blk])
                    nc.tensor.matmul(
                        out=pa[:],
                        lhsT=g[gh][:, gidx, :IN_CH],
                        rhs=S[:, j, :],
                        start=(j == 0),
                        stop=(j == Tb - 1),
                    )
                h_t = h_p.tile([IN_CH, P], bf16)
                nc.vector.tensor_add(out=h_t[:], in0=pa[:],
                                     in1=xt_t[:, blk * P:(blk + 1) * P])
                pm = psum_mlp.tile([OUT_CH, P], f32, space="PSUM")
                nc.tensor.matmul(out=pm[:], lhsT=wt_t[:], rhs=h_t[:],
                                 start=True, stop=True)
                r_t = h_p.tile([OUT_CH, P], f32, tag="r")
                nc.scalar.activation(out=r_t[:], in_=pm[:],
                                     func=mybir.ActivationFunctionType.Relu,
                                     bias=b_t[:])
                pt = psum_tr.tile([P, OUT_CH], f32, space="PSUM")
                nc.tensor.transpose(out=pt[:], in_=r_t[:], identity=ident_t[:])
                rows = min(P, SHARD - blk * P)
                o_t = o_p.tile([P, OUT_CH], f32)
                nc.scalar.activation(out=o_t[:], in_=pt[:],
                                     func=mybir.ActivationFunctionType.Copy)
                nc.sync.dma_start(out=out_d.ap()[blk * P: blk * P + rows, :],
                                  in_=o_t[:rows, :])

    nc.compile()
    return nc


def _prepare(x, edge_index, W, b):
    """Host-side routing + per-core input maps. Returns (in_maps, route)."""
    f16np = np.float16
    x = np.asarray(x, np.float32)
    W = np.asarray(W, np.float32)
    b = np.asarray(b, np.float32)
    src = np.asarray(edge_index[0])
    dst = np.asarray(edge_index[1])

    r = _route(src, dst)
    TBMAX = int((r["K0"] + r["K1"]).max())

    xg = np.zeros((N_NODES, 128), f16np)
    xg[:, :IN_CH] = x.astype(f16np)
    iota = np.tile(np.arange(P, dtype=np.float32), (P, 1)).astype(f16np)
    ident = np.eye(OUT_CH, dtype=np.float32)
    wt = np.ascontiguousarray(W.T).astype(ml_dtypes.bfloat16)
    b2 = np.ascontiguousarray(b.reshape(-1, 1))

    in_maps = []
    for c in range(N_CORES):
        xt = np.zeros((IN_CH, NBLK * P), np.float32)
        xt[:, :SHARD] = x[c * SHARD:(c + 1) * SHARD].T
        slots = r["slots"][c]
        in_maps.append({
            "xg": xg,
            "i0": _wrap_idx(r["idx0"][c]),
            "i1": _wrap_idx(r["idx1"][c]),
            "s": np.ascontiguousarray(slots.reshape(-1, P).T).astype(f16np),
            "xt": np.ascontiguousarray(xt),
            "wt": wt,
            "b": b2,
            "iota": iota,
            "ident": ident,
        })
    return in_maps, r


_CACHE = {}


def _get_program(K0, K1):
    key = (tuple(K0), tuple(K1))
    if key not in _CACHE:
        _CACHE[key] = _build_program(K0, K1)
    return _CACHE[key]


def _best_effort_device_reset():
    """If a previous process wedged the NeuronCores, a reset lets this
    process's run succeed. Harmless (rc=0, state-free) on a healthy device."""
    try:
        import ctypes, jax
        jax.devices()
        lib = ctypes.CDLL("/opt/axon/libaxon_pjrt.so")
        lib.axon_reset.restype = ctypes.c_int64
        lib.axon_reset()
    except Exception:
        pass


def run(x, edge_index, W, b, trace=False):
    from concourse.bass_utils import run_bass_kernel_spmd
    _best_effort_device_reset()
    in_maps, r = _prepare(x, edge_index, W, b)
    nc = _get_program(r["K0"], r["K1"])
    res = run_bass_kernel_spmd(nc, in_maps, core_ids=list(range(N_CORES)),
                               trace=trace)
    out = np.concatenate([res.results[c]["out"] for c in range(N_CORES)], axis=0)
    return out.astype(np.float32), res


def kernel(x, edge_index, W, b):
    out, _ = run(x, edge_index, W, b, trace=False)
    return out
